# revision 16
# baseline (speedup 1.0000x reference)
"""ETS 'AAA' (additive error/trend/seasonal) recurrence on 8 trn2 NeuronCores.

Reformulation (exact algebra, validated vs the sequential reference):
  With u_t = s_read(t) + e_t, the level/trend recurrence collapses to
      l_{t+1} = l_t + b_t + alpha*u_t
      b_{t+1} = b_t + alpha*beta*u_t
  so with s1 = alpha*(1+beta), s2 = alpha*beta and exclusive cumsums
  C_t = sum_{m<t} u_m, D_t = sum_{k<t} C_k:
      y_t = l0 + (t+1)*b0 + s1*C_t + s2*D_t + s_read(t) + 0.1*obs_t

  The linear part l0+(t+1)*b0 is folded into the scan initial states:
  with c0 = b0/s2 and d0 = (l0 + b0 - s1*c0)/s2, the shifted scans
  C'_t = c0 + C_t and D'_t = d0 + t*c0 + D_t satisfy
      y_t = s1*C'_t + s2*D'_t + s_read(t) + 0.1*obs_t
  exactly. (s2 is clamped at 1e-10; errors introduced by the clamp or by
  rounding of the large initials are scaled back down by s2.)

  s_read(t) is per-slot exponential smoothing (12 independent first-order
  recurrences, slot j visited at t = j, j+12, ...):
      S <- (1-gamma)*S + gamma*e_t   (read value is the pre-update state)

  Engine mapping (series on partitions, time on the free axis):
  - DVE: only the three scan passes (12 strided seasonal scans, the C'
    scan with the u_t add fused in, and the chunk-chained D' scan).
  - ACT: gamma*err and the obs->bf16 convert, plus the final PSUM->SBUF
    copy of y.
  - PE: the entire output assembly as diagonal matmuls (per-partition
    scaling) accumulated in PSUM: sp + 0.1*obs + s1*C' in bf16 (these
    streams are small in magnitude, so bf16 quantization is harmless:
    validated 8.9e-4 absmax-relative vs float64 end to end) and the
    dominant s2*D' term in full fp32.
  No sequential timestep loop anywhere.

Sharding: N=4096 series split across 8 cores (512 each), embarrassingly
parallel; each core runs 4 partition-tiles of 128 series x 4096 timesteps.
"""

import numpy as np

import concourse.bass as bass
import concourse.mybir as mybir
from concourse.bass_utils import run_bass_kernel_spmd
from concourse.tile import TileContext

N, T, P = 4096, 4096, 12
NCORES = 8
NS = N // NCORES          # series per core
PT = NS // 128            # partition tiles per core
CH = 512                  # PSUM bank width (PE matmul chunk)
NCH = T // CH
DCH = 1024                # D'-scan / output chunk width
NDCH = T // DCH
F32 = mybir.dt.float32
BF16 = mybir.dt.bfloat16
ALU = mybir.AluOpType
AF = mybir.ActivationFunctionType


def legalize_waits(nc: bass.Bass, max_embedded: int = 1) -> int:
    """Split multi-wait sync_info into preceding EventSemaphore instructions.

    The walrus build in this container encodes at most one embedded sem-wait
    per non-EventSemaphore instruction (EventSemaphore takes two); Tile's
    sem assignment freely emits more. Hoist the extras onto standalone
    EventSemaphore waits on the same engine, immediately before the
    instruction, which is semantically identical (AND of waits).
    """
    n_new = 0
    for f in nc.m.functions:
        for blk in f.blocks:
            new_insts = []
            for inst in blk.instructions:
                si = inst.sync_info
                waits = list(si.on_wait) if si and si.on_wait else []
                limit = 2 if isinstance(inst, mybir.InstEventSemaphore) else max_embedded
                if len(waits) > limit:
                    extra = waits[:-limit] if limit else waits
                    keep = waits[-limit:] if limit else []
                    for i in range(0, len(extra), 2):
                        chunk = extra[i:i + 2]
                        ev = mybir.InstEventSemaphore(
                            name=f"legalize_wait_{inst.name}_{i}")
                        ev.engine = inst.engine
                        ev.sync_info = mybir.SyncInfo(
                            on_wait=list(chunk), on_update=[])
                        new_insts.append(ev)
                        n_new += 1
                    inst.sync_info = mybir.SyncInfo(
                        on_wait=list(keep),
                        on_update=list(si.on_update) if si.on_update else [])
                new_insts.append(inst)
            blk.instructions = new_insts
    return n_new


def build_bass(legalize: bool = True) -> bass.Bass:
    nc = bass.Bass()

    g_alpha = nc.dram_tensor("alpha", [NS], F32, kind="ExternalInput")
    g_beta = nc.dram_tensor("beta", [NS], F32, kind="ExternalInput")
    g_gamma = nc.dram_tensor("gamma", [NS], F32, kind="ExternalInput")
    g_l0 = nc.dram_tensor("init_level", [NS], F32, kind="ExternalInput")
    g_b0 = nc.dram_tensor("init_trend", [NS], F32, kind="ExternalInput")
    g_s0 = nc.dram_tensor("init_seasonal", [NS, P], F32, kind="ExternalInput")
    g_obs = nc.dram_tensor("obs_noise", [NS, T], F32, kind="ExternalInput")
    g_err = nc.dram_tensor("err", [NS, T], F32, kind="ExternalInput")
    g_y = nc.dram_tensor("y", [NS, T], F32, kind="ExternalOutput")

    # visits per seasonal slot j: t = j, j+12, ... < T
    K = [(T - j + P - 1) // P for j in range(P)]

    with TileContext(nc) as tc:
        with (
            tc.tile_pool(name="const", bufs=1) as cpool,
            tc.tile_pool(name="io", bufs=2) as io,
            tc.tile_pool(name="wk", bufs=2) as wk,
            tc.tile_pool(name="ps", bufs=1, space="PSUM") as pspool,
        ):
            # ---- per-series parameters, all tiles at once: [128, PT] ----
            al = cpool.tile([128, PT], F32)
            be = cpool.tile([128, PT], F32)
            ga = cpool.tile([128, PT], F32)
            l0 = cpool.tile([128, PT], F32)
            b0 = cpool.tile([128, PT], F32)
            s0 = cpool.tile([128, PT, P], F32)
            for t, g in ((al, g_alpha), (be, g_beta), (ga, g_gamma),
                         (l0, g_l0), (b0, g_b0)):
                nc.gpsimd.dma_start(t[:], g.rearrange("(t p) -> p t", p=128))
            nc.gpsimd.dma_start(s0[:], g_s0.rearrange("(t p) j -> p t j", p=128))

            # derived per-series scalars, all tiles at once
            s2 = cpool.tile([128, PT], F32)      # alpha*beta
            s1 = cpool.tile([128, PT], F32)      # alpha*(1+beta)
            omg = cpool.tile([128, PT], F32)     # 1-gamma
            r2 = cpool.tile([128, PT], F32)      # 1/max(s2,1e-10)
            c0 = cpool.tile([128, PT], F32)      # b0/s2
            d0 = cpool.tile([128, PT], F32)      # (l0+b0-s1*c0)/s2
            tm = cpool.tile([128, PT], F32)
            nc.vector.tensor_tensor(s2[:], al[:], be[:], ALU.mult)
            nc.vector.tensor_tensor(s1[:], al[:], s2[:], ALU.add)
            nc.vector.tensor_scalar(omg[:], ga[:], -1.0, 1.0, ALU.mult, ALU.add)
            nc.vector.tensor_scalar(r2[:], s2[:], 1e-10, None, ALU.max)
            nc.vector.reciprocal(r2[:], r2[:])
            nc.vector.tensor_tensor(c0[:], b0[:], r2[:], ALU.mult)
            nc.vector.tensor_tensor(tm[:], s1[:], c0[:], ALU.mult)
            nc.vector.tensor_tensor(d0[:], l0[:], b0[:], ALU.add)
            nc.vector.tensor_tensor(d0[:], d0[:], tm[:], ALU.subtract)
            nc.vector.tensor_tensor(d0[:], d0[:], r2[:], ALU.mult)

            # constant diagonal masks for the PE assembly matmuls
            ones_t = cpool.tile([128, 128], F32)
            diag1f = cpool.tile([128, 128], F32)
            diag01f = cpool.tile([128, 128], F32)
            nc.vector.memset(ones_t[:], 1.0)
            nc.gpsimd.affine_select(
                diag1f[:], ones_t[:], pattern=[[1, 128]],
                compare_op=ALU.is_equal, fill=0.0, base=0,
                channel_multiplier=-1)
            nc.vector.tensor_scalar(diag01f[:], diag1f[:], 0.1, None, ALU.mult)

            for i in range(PT):
                sl = slice(i * 128, (i + 1) * 128)

                err_t = io.tile([128, T], F32, tag="err")
                obs_t = io.tile([128, T], F32, tag="obs")
                ge_t = io.tile([128, T], F32, tag="ge", bufs=3)
                for h in range(2):
                    hs = slice(h * (T // 2), (h + 1) * (T // 2))
                    nc.sync.dma_start(err_t[:, hs], g_err[sl, hs])
                    # ge = gamma * err  (feeds the seasonal scans)
                    nc.scalar.activation(ge_t[:, hs], err_t[:, hs], AF.Copy,
                                         scale=ga[:, i: i + 1])
                nc.sync.dma_start(obs_t[:], g_obs[sl, :])
                # (1-gamma) broadcast along free for scan data0 (ACT fill)
                omg_b = wk.tile([128, 512], F32, tag="omgb")
                nc.scalar.activation(omg_b[:], err_t[:, 0:512], AF.Identity,
                                     scale=0.0, bias=omg[:, i: i + 1])

                # seasonal pre-update values s_read(t)
                sp_t = io.tile([128, T], F32, tag="sp")
                nc.vector.tensor_copy(sp_t[:, 0:P], s0[:, i, :])
                for j in range(P):
                    cnt = K[j] - 1
                    nc.vector.tensor_tensor_scan(
                        sp_t[:, j + P:: P][:, :cnt],
                        omg_b[:, 0:cnt],
                        ge_t[:, j:: P][:, :cnt],
                        s0[:, i, j: j + 1],
                        ALU.mult,
                        ALU.add,
                    )

                # C' = c0 + exclusive cumsum(sp + err) -> overwrite ge_t
                nc.vector.tensor_copy(ge_t[:, 0:1], c0[:, i: i + 1])
                nc.vector.tensor_tensor_scan(
                    ge_t[:, 1:T], sp_t[:, 0: T - 1], err_t[:, 0: T - 1],
                    c0[:, i: i + 1], ALU.add, ALU.add,
                )

                # per-tile scaled diagonal
                diag_s1 = wk.tile([128, 128], F32, tag="diag_s1")
                nc.vector.tensor_tensor(
                    diag_s1[:], diag1f[:],
                    s1[:, i: i + 1].broadcast_to([128, 128]), ALU.mult)

                # PE: psum = sp + 0.1*obs + s1*C'   (fp32 diag matmuls)
                # two PSUM half-tiles so tile i+1's matmuls don't wait on
                # tile i's full output drain
                HB = T // 2
                ps0 = pspool.tile([128, HB], F32, tag="ps")
                ps1 = pspool.tile([128, HB], F32, tag="ps")
                halves = (ps0, ps1)
                for d, (w, src) in enumerate(
                        [(diag1f, sp_t), (diag01f, obs_t), (diag_s1, ge_t)]):
                    for c in range(NCH):
                        ph = halves[(c * CH) // HB]
                        off = (c * CH) % HB
                        nc.tensor.matmul(
                            ph[:, off: off + CH], w[:],
                            src[:, c * CH:(c + 1) * CH],
                            start=(d == 0), stop=(d == 2))

                # D' = d0 + exclusive cumsum(C') -> err_t, chunk-chained so
                # the assembly/store tail pipelines per chunk
                nc.vector.tensor_copy(err_t[:, 0:1], d0[:, i: i + 1])
                for c in range(NDCH):
                    lo = 1 + c * DCH
                    hi = min(1 + (c + 1) * DCH, T)
                    init = d0[:, i: i + 1] if c == 0 else err_t[:, c * DCH: c * DCH + 1]
                    nc.vector.tensor_tensor_scan(
                        err_t[:, lo:hi],
                        ge_t[:, lo - 1: hi - 1], ge_t[:, lo - 1: hi - 1],
                        init, ALU.add, ALU.bypass,
                    )
                    # y chunk = s2*D' + psum (exact fp32, DVE), then store
                    ph = halves[(c * DCH) // HB]
                    off = (c * DCH) % HB
                    nc.vector.scalar_tensor_tensor(
                        obs_t[:, c * DCH:(c + 1) * DCH],
                        err_t[:, c * DCH:(c + 1) * DCH],
                        s2[:, i: i + 1],
                        ph[:, off: off + DCH],
                        ALU.mult, ALU.add)
                    nc.sync.dma_start(g_y[sl, c * DCH:(c + 1) * DCH],
                                      obs_t[:, c * DCH:(c + 1) * DCH])

    if legalize:
        legalize_waits(nc)
    return nc


def _shard_inputs(inputs: dict) -> list[dict]:
    in_maps = []
    for c in range(NCORES):
        sl = slice(c * NS, (c + 1) * NS)
        m = {}
        for k in ("alpha", "beta", "gamma", "init_level", "init_trend",
                  "init_seasonal", "obs_noise", "err"):
            m[k] = np.ascontiguousarray(np.asarray(inputs[k], dtype=np.float32)[sl])
        in_maps.append(m)
    return in_maps


def run(inputs: dict, trace: bool = False):
    nc = build_bass()
    in_maps = _shard_inputs(inputs)
    res = run_bass_kernel_spmd(nc, in_maps, core_ids=list(range(NCORES)),
                               trace=trace)
    y = np.concatenate([res.results[c]["y"] for c in range(NCORES)], axis=0)
    return y, res


def kernel(**inputs) -> np.ndarray:
    y, _ = run(inputs)
    return y


# revision 17
# speedup vs baseline: 1.1479x; 1.1479x over previous
"""ETS 'AAA' (additive error/trend/seasonal) recurrence on 8 trn2 NeuronCores.

Reformulation (exact algebra, validated vs the sequential reference):
  With u_t = s_read(t) + e_t, the level/trend recurrence collapses to
      l_{t+1} = l_t + b_t + alpha*u_t
      b_{t+1} = b_t + alpha*beta*u_t
  so with s1 = alpha*(1+beta), s2 = alpha*beta and exclusive cumsums
  C_t = sum_{m<t} u_m, D_t = sum_{k<t} C_k:
      y_t = l0 + (t+1)*b0 + s1*C_t + s2*D_t + s_read(t) + 0.1*obs_t

  The linear part l0+(t+1)*b0 is folded into the scan initial states:
  with c0 = b0/s2 and d0 = (l0 + b0 - s1*c0)/s2, the shifted scans
  C'_t = c0 + C_t and D'_t = d0 + t*c0 + D_t satisfy
      y_t = s1*C'_t + s2*D'_t + s_read(t) + 0.1*obs_t
  exactly. (s2 is clamped at 1e-10; errors introduced by the clamp or by
  rounding of the large initials are scaled back down by s2.)

  s_read(t) is per-slot exponential smoothing (12 independent first-order
  recurrences, slot j visited at t = j, j+12, ...):
      S <- (1-gamma)*S + gamma*e_t   (read value is the pre-update state)

  Engine mapping (series on partitions, time on the free axis):
  - DVE: only the three scan passes (12 strided seasonal scans, the C'
    scan with the u_t add fused in, and the chunk-chained D' scan).
  - ACT: gamma*err and the obs->bf16 convert, plus the final PSUM->SBUF
    copy of y.
  - PE: the entire output assembly as diagonal matmuls (per-partition
    scaling) accumulated in PSUM: sp + 0.1*obs + s1*C' in bf16 (these
    streams are small in magnitude, so bf16 quantization is harmless:
    validated 8.9e-4 absmax-relative vs float64 end to end) and the
    dominant s2*D' term in full fp32.
  No sequential timestep loop anywhere.

Sharding: N=4096 series split across 8 cores (512 each), embarrassingly
parallel; each core runs 4 partition-tiles of 128 series x 4096 timesteps.
"""

import numpy as np

import concourse.bass as bass
import concourse.mybir as mybir
from concourse.bass_utils import run_bass_kernel_spmd
from concourse.tile import TileContext

N, T, P = 4096, 4096, 12
NCORES = 8
NS = N // NCORES          # series per core
PT = NS // 128            # partition tiles per core
CH = 512                  # PSUM bank width (PE matmul chunk)
NCH = T // CH
DCH = 1024                # D'-scan / output chunk width
NDCH = T // DCH
F32 = mybir.dt.float32
BF16 = mybir.dt.bfloat16
ALU = mybir.AluOpType
AF = mybir.ActivationFunctionType


def legalize_waits(nc: bass.Bass, max_embedded: int = 1) -> int:
    """Split multi-wait sync_info into preceding EventSemaphore instructions.

    The walrus build in this container encodes at most one embedded sem-wait
    per non-EventSemaphore instruction (EventSemaphore takes two); Tile's
    sem assignment freely emits more. Hoist the extras onto standalone
    EventSemaphore waits on the same engine, immediately before the
    instruction, which is semantically identical (AND of waits).
    """
    n_new = 0
    for f in nc.m.functions:
        for blk in f.blocks:
            new_insts = []
            for inst in blk.instructions:
                si = inst.sync_info
                waits = list(si.on_wait) if si and si.on_wait else []
                limit = 2 if isinstance(inst, mybir.InstEventSemaphore) else max_embedded
                if len(waits) > limit:
                    extra = waits[:-limit] if limit else waits
                    keep = waits[-limit:] if limit else []
                    for i in range(0, len(extra), 2):
                        chunk = extra[i:i + 2]
                        ev = mybir.InstEventSemaphore(
                            name=f"legalize_wait_{inst.name}_{i}")
                        ev.engine = inst.engine
                        ev.sync_info = mybir.SyncInfo(
                            on_wait=list(chunk), on_update=[])
                        new_insts.append(ev)
                        n_new += 1
                    inst.sync_info = mybir.SyncInfo(
                        on_wait=list(keep),
                        on_update=list(si.on_update) if si.on_update else [])
                new_insts.append(inst)
            blk.instructions = new_insts
    return n_new


def build_bass(legalize: bool = True) -> bass.Bass:
    nc = bass.Bass()

    g_alpha = nc.dram_tensor("alpha", [NS], F32, kind="ExternalInput")
    g_beta = nc.dram_tensor("beta", [NS], F32, kind="ExternalInput")
    g_gamma = nc.dram_tensor("gamma", [NS], F32, kind="ExternalInput")
    g_l0 = nc.dram_tensor("init_level", [NS], F32, kind="ExternalInput")
    g_b0 = nc.dram_tensor("init_trend", [NS], F32, kind="ExternalInput")
    g_s0 = nc.dram_tensor("init_seasonal", [NS, P], F32, kind="ExternalInput")
    g_obs = nc.dram_tensor("obs_noise", [NS, T], F32, kind="ExternalInput")
    g_err = nc.dram_tensor("err", [NS, T], F32, kind="ExternalInput")
    g_y = nc.dram_tensor("y", [NS, T], F32, kind="ExternalOutput")

    # visits per seasonal slot j: t = j, j+12, ... < T
    K = [(T - j + P - 1) // P for j in range(P)]

    with TileContext(nc) as tc:
        with (
            tc.tile_pool(name="const", bufs=1) as cpool,
            tc.tile_pool(name="io", bufs=2) as io,
            tc.tile_pool(name="wk", bufs=2) as wk,
            tc.tile_pool(name="ps", bufs=1, space="PSUM") as pspool,
        ):
            # ---- per-series parameters, all tiles at once: [128, PT] ----
            al = cpool.tile([128, PT], F32)
            be = cpool.tile([128, PT], F32)
            ga = cpool.tile([128, PT], F32)
            l0 = cpool.tile([128, PT], F32)
            b0 = cpool.tile([128, PT], F32)
            s0 = cpool.tile([128, PT, P], F32)
            for t, g in ((al, g_alpha), (be, g_beta), (ga, g_gamma),
                         (l0, g_l0), (b0, g_b0)):
                nc.gpsimd.dma_start(t[:], g.rearrange("(t p) -> p t", p=128))
            nc.gpsimd.dma_start(s0[:], g_s0.rearrange("(t p) j -> p t j", p=128))

            # derived per-series scalars, all tiles at once
            s2 = cpool.tile([128, PT], F32)      # alpha*beta
            s1 = cpool.tile([128, PT], F32)      # alpha*(1+beta)
            omg = cpool.tile([128, PT], F32)     # 1-gamma
            r2 = cpool.tile([128, PT], F32)      # 1/max(s2,1e-10)
            c0 = cpool.tile([128, PT], F32)      # b0/s2
            d0 = cpool.tile([128, PT], F32)      # (l0+b0-s1*c0)/s2
            tm = cpool.tile([128, PT], F32)
            nc.vector.tensor_tensor(s2[:], al[:], be[:], ALU.mult)
            nc.vector.tensor_tensor(s1[:], al[:], s2[:], ALU.add)
            nc.vector.tensor_scalar(omg[:], ga[:], -1.0, 1.0, ALU.mult, ALU.add)
            nc.vector.tensor_scalar(r2[:], s2[:], 1e-10, None, ALU.max)
            nc.vector.reciprocal(r2[:], r2[:])
            nc.vector.tensor_tensor(c0[:], b0[:], r2[:], ALU.mult)
            nc.vector.tensor_tensor(tm[:], s1[:], c0[:], ALU.mult)
            nc.vector.tensor_tensor(d0[:], l0[:], b0[:], ALU.add)
            nc.vector.tensor_tensor(d0[:], d0[:], tm[:], ALU.subtract)
            nc.vector.tensor_tensor(d0[:], d0[:], r2[:], ALU.mult)

            # constant diagonal masks for the PE assembly matmuls
            ones_t = cpool.tile([128, 128], F32)
            diag1f = cpool.tile([128, 128], F32)
            diag01f = cpool.tile([128, 128], F32)
            nc.vector.memset(ones_t[:], 1.0)
            nc.gpsimd.affine_select(
                diag1f[:], ones_t[:], pattern=[[1, 128]],
                compare_op=ALU.is_equal, fill=0.0, base=0,
                channel_multiplier=-1)
            nc.vector.tensor_scalar(diag01f[:], diag1f[:], 0.1, None, ALU.mult)

            for i in range(PT):
                sl = slice(i * 128, (i + 1) * 128)

                err_t = io.tile([128, T], F32, tag="err")
                obs_t = io.tile([128, T], F32, tag="obs")
                nc.sync.dma_start(err_t[:], g_err[sl, :])
                nc.sync.dma_start(obs_t[:], g_obs[sl, :])

                # ge = gamma * err  (feeds the seasonal scans)
                ge_t = io.tile([128, T], F32, tag="ge", bufs=3)
                nc.scalar.activation(ge_t[:], err_t[:], AF.Copy,
                                     scale=ga[:, i: i + 1])
                # (1-gamma) broadcast along free for scan data0 (ACT fill)
                omg_b = wk.tile([128, 512], F32, tag="omgb")
                nc.scalar.activation(omg_b[:], err_t[:, 0:512], AF.Identity,
                                     scale=0.0, bias=omg[:, i: i + 1])

                # seasonal pre-update values s_read(t)
                sp_t = io.tile([128, T], F32, tag="sp")
                nc.vector.tensor_copy(sp_t[:, 0:P], s0[:, i, :])
                for j in range(P):
                    cnt = K[j] - 1
                    nc.vector.tensor_tensor_scan(
                        sp_t[:, j + P:: P][:, :cnt],
                        omg_b[:, 0:cnt],
                        ge_t[:, j:: P][:, :cnt],
                        s0[:, i, j: j + 1],
                        ALU.mult,
                        ALU.add,
                    )

                # C' = c0 + exclusive cumsum(sp + err) -> overwrite ge_t
                nc.vector.tensor_copy(ge_t[:, 0:1], c0[:, i: i + 1])
                nc.vector.tensor_tensor_scan(
                    ge_t[:, 1:T], sp_t[:, 0: T - 1], err_t[:, 0: T - 1],
                    c0[:, i: i + 1], ALU.add, ALU.add,
                )

                # per-tile scaled diagonal
                diag_s1 = wk.tile([128, 128], F32, tag="diag_s1")
                nc.vector.tensor_tensor(
                    diag_s1[:], diag1f[:],
                    s1[:, i: i + 1].broadcast_to([128, 128]), ALU.mult)

                # PE: psum = sp + 0.1*obs + s1*C'   (fp32 diag matmuls)
                ps = pspool.tile([128, T], F32, tag="ps")
                for d, (w, src) in enumerate(
                        [(diag1f, sp_t), (diag01f, obs_t), (diag_s1, ge_t)]):
                    for c in range(NCH):
                        nc.tensor.matmul(
                            ps[:, c * CH:(c + 1) * CH], w[:],
                            src[:, c * CH:(c + 1) * CH],
                            start=(d == 0), stop=(d == 2))

                # D' = d0 + exclusive cumsum(C') -> err_t, chunk-chained so
                # the assembly/store tail pipelines per chunk
                nc.vector.tensor_copy(err_t[:, 0:1], d0[:, i: i + 1])
                for c in range(NDCH):
                    lo = 1 + c * DCH
                    hi = min(1 + (c + 1) * DCH, T)
                    init = d0[:, i: i + 1] if c == 0 else err_t[:, c * DCH: c * DCH + 1]
                    nc.vector.tensor_tensor_scan(
                        err_t[:, lo:hi],
                        ge_t[:, lo - 1: hi - 1], ge_t[:, lo - 1: hi - 1],
                        init, ALU.add, ALU.bypass,
                    )
                    # y chunk = s2*D' + psum (exact fp32, DVE), then store
                    nc.vector.scalar_tensor_tensor(
                        obs_t[:, c * DCH:(c + 1) * DCH],
                        err_t[:, c * DCH:(c + 1) * DCH],
                        s2[:, i: i + 1],
                        ps[:, c * DCH:(c + 1) * DCH],
                        ALU.mult, ALU.add)
                    nc.sync.dma_start(g_y[sl, c * DCH:(c + 1) * DCH],
                                      obs_t[:, c * DCH:(c + 1) * DCH])

    if legalize:
        legalize_waits(nc)
    return nc


def _shard_inputs(inputs: dict) -> list[dict]:
    in_maps = []
    for c in range(NCORES):
        sl = slice(c * NS, (c + 1) * NS)
        m = {}
        for k in ("alpha", "beta", "gamma", "init_level", "init_trend",
                  "init_seasonal", "obs_noise", "err"):
            m[k] = np.ascontiguousarray(np.asarray(inputs[k], dtype=np.float32)[sl])
        in_maps.append(m)
    return in_maps


def run(inputs: dict, trace: bool = False):
    nc = build_bass()
    in_maps = _shard_inputs(inputs)
    res = run_bass_kernel_spmd(nc, in_maps, core_ids=list(range(NCORES)),
                               trace=trace)
    y = np.concatenate([res.results[c]["y"] for c in range(NCORES)], axis=0)
    return y, res


def kernel(**inputs) -> np.ndarray:
    y, _ = run(inputs)
    return y
